# revision 14
# baseline (speedup 1.0000x reference)
"""AttentionFlow (BiDAF-style) Trainium2 kernel, data-parallel over batch on 8 cores.

Shapes (hardcoded): B=16, Q=64, C=512, D=512; final GEMM [C,4D]@[4D,4D].
Each core handles 2 batches independently; no collectives.

Per-batch math (matching the reference):
  sim'[c,q] = (ctx*w_p)[c,:]@qst[q,:] + qw[q]        (cw[c] cancels in softmax over q)
  S = softmax_q(sim'); u = S @ qst
  attn = softmax_c(max_q sim' + cw);  h = attn @ ctx
  A = [ctx, u, u*ctx, h*ctx]  (built transposed, [4D, C])
  out = A @ qac_w.T + qac_b

qac_w.T is built with TensorE transposes in bf16, column-strip order, with the
first GEMM chunk's matmuls interleaved so the PE stays warm and the GEMM starts
as soon as the first k-tiles of W^T land. cw rides as a 65th rhs column of the
sim matmul; qac_b is broadcast by a stride-0 DMA and added during the PSUM
drain on the vector engine.
"""

import numpy as np

import concourse.bass as bass
import concourse.mybir as mybir
import concourse.tile as tile
from concourse import bacc
from concourse.masks import make_identity

B, Q, C, D = 16, 64, 512, 512
KN = 4 * D  # 2048
NCORES = 8
BPC = B // NCORES  # 2 batches per core

F32 = mybir.dt.float32
BF16 = mybir.dt.bfloat16

MT = C // 128   # 4 c-tiles
DT = D // 128   # 4 d-tiles
KT = KN // 128  # 16 gemm k-tiles
NT = KN // 512  # 4 gemm n-tiles
GEMM_CHUNK = 4  # concurrent PSUM banks for the big GEMM


def _build(tc, nc, qst_d, ctx_d, simw_d, qacw_d, qacb_d, out_d, scratch):
    from contextlib import ExitStack

    Exp = mybir.ActivationFunctionType.Exp
    AX = mybir.AxisListType.X

    with ExitStack() as ctx:
        consts = ctx.enter_context(tc.tile_pool(name="consts", bufs=1))
        wpool = ctx.enter_context(tc.tile_pool(name="wpool", bufs=1))
        stg = ctx.enter_context(tc.tile_pool(name="stg", bufs=1))
        ab = ctx.enter_context(tc.tile_pool(name="ab", bufs=2))      # A blocks (both batches live)
        b1 = ctx.enter_context(tc.tile_pool(name="b1", bufs=1))      # attention-only tiles
        sm = ctx.enter_context(tc.tile_pool(name="sm", bufs=2))      # small per-batch tiles
        ostage = ctx.enter_context(tc.tile_pool(name="ostage", bufs=1))
        psum_og = ctx.enter_context(tc.tile_pool(name="psum_og", bufs=GEMM_CHUNK, space="PSUM"))
        psum_pw = ctx.enter_context(tc.tile_pool(name="psum_pw", bufs=2, space="PSUM"))
        psum_big = ctx.enter_context(tc.tile_pool(name="psum_big", bufs=1, space="PSUM"))
        psum_at = ctx.enter_context(tc.tile_pool(name="psum_at", bufs=1, space="PSUM"))

        # ---------------- constants ----------------
        ident = consts.tile([128, 128], F32)
        make_identity(nc, ident)
        ident_bf = consts.tile([128, 128], BF16)
        make_identity(nc, ident_bf)
        ones_f32 = consts.tile([1, 128], F32)
        nc.vector.memset(ones_f32, 1.0)
        one_one = consts.tile([1, 1], F32)
        nc.vector.memset(one_one, 1.0)

        simw = simw_d.ap()
        wc_all = consts.tile([128, DT], F32)  # w_c as 4 column tiles
        wq_all = consts.tile([128, DT], F32)
        wp_all = consts.tile([128, DT], F32)
        nc.sync.dma_start(out=wc_all, in_=simw[0, 0:D].rearrange("(k p) -> p k", p=128))
        nc.sync.dma_start(out=wq_all, in_=simw[0, D:2 * D].rearrange("(k p) -> p k", p=128))
        nc.sync.dma_start(out=wp_all, in_=simw[0, 2 * D:3 * D].rearrange("(k p) -> p k", p=128))

        # qac_b broadcast to all 128 partitions via stride-0 DMA
        bias_bc = consts.tile([128, KN], F32)
        qb_ap = qacb_d.ap()
        nc.sync.dma_start(
            out=bias_bc,
            in_=bass.AP(tensor=qb_ap.tensor, offset=qb_ap.offset, ap=[[0, 128]] + qb_ap.ap),
        )

        # ---------------- attention for both batches ----------------
        # (emitted before the weight prologue so its PE work runs while
        #  qac_w streams in from HBM)
        ct = {}        # [b,ci] ctx tiles   [128c, 512d] f32
        ctxT_bf = {}   # [b,di] A block 0   [128d, 512c] bf16
        uT_bf = {}     # [b,di] A block 1
        uc_bf = {}     # [b,di] A block 2
        hc_bf = {}     # [b,di] A block 3

        def emit_attention(b):
            # DMA inputs
            qst = sm.tile([64, D], F32, tag="qst", name=f"qst{b}")
            nc.sync.dma_start(out=qst, in_=qst_d.ap()[b])
            for ci in range(MT):
                t = b1.tile([128, D], F32, tag=f"ct{ci}", name=f"ct{b}_{ci}")
                nc.sync.dma_start(out=t, in_=ctx_d.ap()[b, ci * 128:(ci + 1) * 128, :])
                ct[(b, ci)] = t

            # qstT [D, Q] via PE transpose (4 blocks side by side in one bank)
            p_qt = psum_at.tile([128, DT * 64], F32, tag="at", name=f"pqt{b}")
            for di in range(DT):
                nc.tensor.transpose(
                    p_qt[:, di * 64:(di + 1) * 64],
                    qst[:, di * 128:(di + 1) * 128],
                    ident[0:64, 0:64],
                )
            qstT = sm.tile([128, DT, 64], F32, tag="qstT", name=f"qstT{b}")
            nc.vector.tensor_copy(qstT, p_qt.rearrange("p (d q) -> p d q", d=DT))
            qst_bf = sm.tile([64, D], BF16, tag="qst_bf", name=f"qst_bf{b}")
            nc.scalar.copy(qst_bf, qst)

            # rhs of the sim matmul: [wp*qstT | wc] -> [128, DT, 65]
            wpq = sm.tile([128, DT, 65], F32, tag="wpq", name=f"wpq{b}")
            for di in range(DT):
                nc.vector.tensor_scalar_mul(wpq[:, di, 0:64], qstT[:, di, :], wp_all[:, di:di + 1])
                nc.vector.tensor_copy(wpq[:, di, 64:65], wc_all[:, di:di + 1])

            # ctxT via PE transpose; f32 copy for the sim lhsT, bf16 for A block 0
            ctxT_f32 = {}
            for di in range(DT):
                p_ct = psum_big.tile([128, C], F32, tag="big", name=f"pct{b}_{di}")
                for ci in range(MT):
                    nc.tensor.transpose(
                        p_ct[:, ci * 128:(ci + 1) * 128],
                        ct[(b, ci)][:, di * 128:(di + 1) * 128],
                        ident,
                    )
                tf = b1.tile([128, C], F32, tag=f"ctxT{di}", name=f"ctxT{b}_{di}")
                nc.vector.tensor_copy(tf, p_ct)
                tb = ab.tile([128, C], BF16, tag=f"ctxTb{di}", name=f"ctxTb{b}_{di}")
                nc.vector.tensor_copy(tb, tf)
                ctxT_f32[di] = tf
                ctxT_bf[(b, di)] = tb

            # qw row: qw[q] = qst[q,:] @ w_q  (65th col stays 0 so cw is untouched)
            p_qw = psum_at.tile([1, 64], F32, tag="at", name=f"pqw{b}")
            for di in range(DT):
                nc.tensor.matmul(
                    p_qw, wq_all[:, di:di + 1], qstT[:, di, :],
                    start=(di == 0), stop=(di == DT - 1),
                )
            qw_row = sm.tile([1, 65], F32, tag="qw_row", name=f"qw_row{b}")
            nc.vector.memset(qw_row[0:1, 64:65], 0.0)
            nc.vector.tensor_copy(qw_row[0:1, 0:64], p_qw)

            # sim' cols 0:64 (+ cw in col 64) = ctxT.T @ [wp*qstT | wc] + [qw | 0]
            p_sim = psum_at.tile([128, MT, 65], F32, tag="at", name=f"psim{b}")
            for mi in range(MT):
                for di in range(DT):
                    nc.tensor.matmul(
                        p_sim[:, mi, :],
                        ctxT_f32[di][:, mi * 128:(mi + 1) * 128],
                        wpq[:, di, :],
                        start=(di == 0), stop=False,
                    )
                nc.tensor.matmul(
                    p_sim[:, mi, :], ones_f32, qw_row,
                    start=False, stop=True,
                )

            # row softmax over q (free axis); keep neg-rowmax for the attn path
            negm = sm.tile([128, MT], F32, tag="negm", name=f"negm{b}")
            ssum = sm.tile([128, MT], F32, tag="ssum", name=f"ssum{b}")
            rsum = sm.tile([128, MT], F32, tag="rsum", name=f"rsum{b}")
            S_n = sm.tile([128, MT, 64], F32, tag="S_n", name=f"S_n{b}")
            g_col = sm.tile([128, MT], F32, tag="g_col", name=f"g_col{b}")
            for mi in range(MT):
                nc.vector.reduce_max(negm[:, mi:mi + 1], p_sim[:, mi, 0:64], axis=AX, negate=True)
                nc.scalar.activation(
                    S_n[:, mi, :], p_sim[:, mi, 0:64], Exp,
                    bias=negm[:, mi:mi + 1], accum_out=ssum[:, mi:mi + 1],
                )
                # g[c] = max_q sim' + cw = cw - negm   (cw lives in sim col 64)
                nc.vector.tensor_sub(g_col[:, mi:mi + 1], p_sim[:, mi, 64:65], negm[:, mi:mi + 1])
            nc.vector.reciprocal(rsum, ssum)
            for mi in range(MT):
                nc.vector.tensor_scalar_mul(S_n[:, mi, :], S_n[:, mi, :], rsum[:, mi:mi + 1])

            # S_T [Q, C] (bf16) then u_T = qst.T @ S_T  -> [D, C]
            p_st = psum_at.tile([64, C], F32, tag="at", name=f"pst{b}")
            for mi in range(MT):
                nc.tensor.transpose(
                    p_st[:, mi * 128:(mi + 1) * 128], S_n[:, mi, :], ident,
                )
            S_T = sm.tile([64, C], BF16, tag="S_T", name=f"S_T{b}")
            nc.vector.tensor_copy(S_T, p_st)

            for di in range(DT):
                p_u = psum_big.tile([128, C], F32, tag="big", name=f"pu{b}_{di}")
                nc.tensor.matmul(
                    p_u, qst_bf[:, di * 128:(di + 1) * 128], S_T,
                    start=True, stop=True,
                )
                tu = ab.tile([128, C], BF16, tag=f"uT{di}", name=f"uT{b}_{di}")
                nc.vector.tensor_copy(tu, p_u)
                uT_bf[(b, di)] = tu
                tuc = ab.tile([128, C], BF16, tag=f"uc{di}", name=f"uc{b}_{di}")
                nc.vector.tensor_mul(tuc, tu, ctxT_bf[(b, di)])
                uc_bf[(b, di)] = tuc

            # attn = softmax_c(g) on one partition
            p_g = psum_at.tile([1, C], F32, tag="at", name=f"pg{b}")
            for mi in range(MT):
                nc.tensor.transpose(
                    p_g[0:1, mi * 128:(mi + 1) * 128], g_col[:, mi:mi + 1], ident,
                )
            g_row = sm.tile([1, C], F32, tag="g_row", name=f"g_row{b}", bufs=1)
            nc.vector.tensor_copy(g_row, p_g)
            negm1 = sm.tile([1, 1], F32, tag="negm1", name=f"negm1{b}")
            ssum1 = sm.tile([1, 1], F32, tag="ssum1", name=f"ssum1{b}")
            nc.vector.reduce_max(negm1, g_row, axis=AX, negate=True)
            attn_row = sm.tile([1, C], F32, tag="attn_row", name=f"attn_row{b}", bufs=1)
            nc.scalar.activation(attn_row, g_row, Exp, bias=negm1, accum_out=ssum1)
            rsum1 = sm.tile([1, 1], F32, tag="rsum1", name=f"rsum1{b}")
            nc.vector.reciprocal(rsum1, ssum1)
            nc.vector.tensor_scalar_mul(attn_row, attn_row, rsum1)

            # attn column [C,1]; h_row = attn @ ctx; h column [D,1]
            p_ac = psum_at.tile([128, MT], F32, tag="at", name=f"pac{b}")
            for mi in range(MT):
                nc.tensor.matmul(
                    p_ac[:, mi:mi + 1],
                    attn_row[0:1, mi * 128:(mi + 1) * 128], one_one,
                    start=True, stop=True,
                )
            ac = sm.tile([128, MT], F32, tag="ac", name=f"ac{b}")
            nc.vector.tensor_copy(ac, p_ac)

            p_hr = psum_at.tile([1, D], F32, tag="at", name=f"phr{b}")
            for ci in range(MT):
                nc.tensor.matmul(
                    p_hr, ac[:, ci:ci + 1], ct[(b, ci)],
                    start=(ci == 0), stop=(ci == MT - 1),
                )
            h_row = sm.tile([1, D], F32, tag="h_row", name=f"h_row{b}", bufs=1)
            nc.vector.tensor_copy(h_row, p_hr)

            p_hc = psum_at.tile([128, DT], F32, tag="at", name=f"phc{b}")
            for di in range(DT):
                nc.tensor.matmul(
                    p_hc[:, di:di + 1],
                    h_row[0:1, di * 128:(di + 1) * 128], one_one,
                    start=True, stop=True,
                )
            h_col = sm.tile([128, DT], F32, tag="h_col", name=f"h_col{b}")
            nc.vector.tensor_copy(h_col, p_hc)

            for di in range(DT):
                th = ab.tile([128, C], BF16, tag=f"hc{di}", name=f"hc{b}_{di}")
                nc.vector.tensor_scalar_mul(th, ctxT_bf[(b, di)], h_col[:, di:di + 1])
                hc_bf[(b, di)] = th

        # -------- qac_w.T prologue (column strips, PE transposes in bf16) --------
        # interleaved with GEMM chunk 0 so the PE warms up and the GEMM starts as
        # soon as the first k-tiles of W^T are ready.
        WT = [wpool.tile([128, KN], BF16, tag=f"wt{k}", name=f"wt{k}") for k in range(KT)]

        # one chunk per (batch, c-tile): the 4 matmuls of each k-layer share the
        # same stationary operand (one LDWEIGHTS per layer), and the drain is a
        # single contiguous [128, 2048] store.
        groups = [(b, mi) for b in range(BPC) for mi in range(MT)]

        def ablk(b, kt):
            if kt < 4:
                return ctxT_bf[(b, kt)]
            if kt < 8:
                return uT_bf[(b, kt - 4)]
            if kt < 12:
                return uc_bf[(b, kt - 8)]
            return hc_bf[(b, kt - 12)]

        def gemm_layer(bmi, psums, kt):
            b, mi = bmi
            for ni in range(NT):
                nc.tensor.matmul(
                    psums[ni],
                    ablk(b, kt)[:, mi * 128:(mi + 1) * 128],
                    WT[kt][:, ni * 512:(ni + 1) * 512],
                    start=(kt == 0), stop=(kt == KT - 1),
                )

        def gemm_drain(bmi, psums, c0):
            b, mi = bmi
            st = ostage.tile([128, KN], F32, tag="ost", name=f"ost{c0}")
            for ni in range(NT):
                nc.vector.tensor_add(
                    st[:, ni * 512:(ni + 1) * 512], psums[ni],
                    bias_bc[:, ni * 512:(ni + 1) * 512],
                )
            nc.sync.dma_start(out=out_d.ap()[b, mi * 128:(mi + 1) * 128, :], in_=st)

        chunk0 = groups[0]
        psums0 = [
            psum_og.tile([128, 512], F32, tag="og", name=f"po0_{gi}")
            for gi in range(NT)
        ]

        # Column halves: load [128,1024] f32 strips of qac_w, cast to a bf16
        # row cache, transpose in a dense burst. Phase order puts batch-0
        # attention before half 0 and batch-1 attention between the halves so
        # every engine has fill-in work while qac_w streams from HBM.
        HW = KN // 2

        def qac_load_half(half):
            rbf = []
            for ri in range(KT):
                t = stg.tile([128, HW], F32, tag=f"qs{ri % 3}", name=f"qs{half}_{ri}")
                nc.sync.dma_start(
                    out=t,
                    in_=qacw_d.ap()[ri * 128:(ri + 1) * 128, half * HW:(half + 1) * HW],
                )
                tb = stg.tile([128, HW], BF16, tag=f"qsb{ri}", name=f"qsb{half}_{ri}")
                nc.vector.tensor_copy(tb, t)
                rbf.append(tb)
            return rbf

        def qac_transpose_half(half, rbf):
            for ktl in range(8):
                kt = half * 8 + ktl
                for rg in range(4):
                    p_w = psum_pw.tile([128, 512], BF16, tag="pw", name=f"pw{kt}_{rg}")
                    for rr in range(4):
                        nc.tensor.transpose(
                            p_w[:, rr * 128:(rr + 1) * 128],
                            rbf[rg * 4 + rr][:, ktl * 128:(ktl + 1) * 128],
                            ident_bf,
                        )
                    if rg % 2 == 0:
                        nc.vector.tensor_copy(WT[kt][:, rg * 512:(rg + 1) * 512], p_w)
                    else:
                        nc.scalar.copy(WT[kt][:, rg * 512:(rg + 1) * 512], p_w)

        emit_attention(0)
        rbf0 = qac_load_half(0)
        qac_transpose_half(0, rbf0)
        for ktl in range(8):
            gemm_layer(chunk0, psums0, ktl)
        emit_attention(1)
        # half 1: bounce bf16 through DRAM and XBAR-transpose on the ACT HWDGE
        # ring (keeps the PE free for GEMM + batch-1 attention).
        rbf1 = qac_load_half(1)
        for ri in range(KT):
            nc.sync.dma_start(
                out=scratch.ap()[ri * 128:(ri + 1) * 128, :], in_=rbf1[ri]
            )
        for kt in range(8, KT):
            nc.scalar.dma_start_transpose(
                out=WT[kt], in_=scratch.ap()[:, (kt - 8) * 128:(kt - 7) * 128]
            )
        for ktl in range(8):
            gemm_layer(chunk0, psums0, 8 + ktl)
        gemm_drain(chunk0, psums0, 0)

        # ---------------- remaining GEMM chunks ----------------
        for c0 in range(1, len(groups)):
            chunk = groups[c0]
            psums = [
                psum_og.tile([128, 512], F32, tag="og", name=f"po{c0}_{gi}")
                for gi in range(NT)
            ]
            for kt in range(KT):
                gemm_layer(chunk, psums, kt)
            gemm_drain(chunk, psums, c0)


def build_nc():
    nc = bacc.Bacc("TRN2", target_bir_lowering=False, debug=False, num_devices=NCORES)
    qst_d = nc.declare_dram_parameter("questions", [BPC, Q, D], F32, isOutput=False)
    ctx_d = nc.declare_dram_parameter("contexts", [BPC, C, D], F32, isOutput=False)
    simw_d = nc.declare_dram_parameter("sim_w", [1, 3 * D], F32, isOutput=False)
    qacw_d = nc.declare_dram_parameter("qac_w", [KN, KN], F32, isOutput=False)
    qacb_d = nc.declare_dram_parameter("qac_b", [KN], F32, isOutput=False)
    out_d = nc.declare_dram_parameter("out", [BPC, C, KN], F32, isOutput=True)
    scratch = nc.dram_tensor("qac_bf16_h1", [KN, KN // 2], mybir.dt.bfloat16)
    with tile.TileContext(nc) as tc:
        _build(tc, nc, qst_d, ctx_d, simw_d, qacw_d, qacb_d, out_d, scratch)
    nc.compile()
    return nc


def shard_inputs(questions, contexts, sim_w, qac_w, qac_b):
    questions = np.ascontiguousarray(np.asarray(questions, dtype=np.float32))
    contexts = np.ascontiguousarray(np.asarray(contexts, dtype=np.float32))
    sim_w = np.ascontiguousarray(np.asarray(sim_w, dtype=np.float32))
    qac_w = np.ascontiguousarray(np.asarray(qac_w, dtype=np.float32))
    qac_b = np.ascontiguousarray(np.asarray(qac_b, dtype=np.float32))
    return [
        {
            "questions": questions[i * BPC:(i + 1) * BPC],
            "contexts": contexts[i * BPC:(i + 1) * BPC],
            "sim_w": sim_w,
            "qac_w": qac_w,
            "qac_b": qac_b,
        }
        for i in range(NCORES)
    ]


_NC = None


def get_nc():
    global _NC
    if _NC is None:
        _NC = build_nc()
    return _NC


def kernel(questions, contexts, sim_w, qac_w, qac_b):
    from concourse.bass_utils import run_bass_kernel_spmd

    nc = get_nc()
    in_maps = shard_inputs(questions, contexts, sim_w, qac_w, qac_b)
    res = run_bass_kernel_spmd(nc, in_maps, core_ids=list(range(NCORES)))
    return np.concatenate([res.results[i]["out"] for i in range(NCORES)], axis=0)


# revision 15
# speedup vs baseline: 1.0934x; 1.0934x over previous
"""AttentionFlow (BiDAF-style) Trainium2 kernel, data-parallel over batch on 8 cores.

Shapes (hardcoded): B=16, Q=64, C=512, D=512; final GEMM [C,4D]@[4D,4D].
Each core handles 2 batches independently; no collectives.

Per-batch math (matching the reference):
  sim'[c,q] = (ctx*w_p)[c,:]@qst[q,:] + qw[q]        (cw[c] cancels in softmax over q)
  S = softmax_q(sim'); u = S @ qst
  attn = softmax_c(max_q sim' + cw);  h = attn @ ctx
  A = [ctx, u, u*ctx, h*ctx]  (built transposed, [4D, C])
  out = A @ qac_w.T + qac_b

qac_w.T is built with TensorE transposes in bf16, column-strip order, with the
first GEMM chunk's matmuls interleaved so the PE stays warm and the GEMM starts
as soon as the first k-tiles of W^T land. cw rides as a 65th rhs column of the
sim matmul; qac_b is broadcast by a stride-0 DMA and added during the PSUM
drain on the vector engine.
"""

import numpy as np

import concourse.bass as bass
import concourse.mybir as mybir
import concourse.tile as tile
from concourse import bacc
from concourse.masks import make_identity

B, Q, C, D = 16, 64, 512, 512
KN = 4 * D  # 2048
NCORES = 8
BPC = B // NCORES  # 2 batches per core

F32 = mybir.dt.float32
BF16 = mybir.dt.bfloat16

MT = C // 128   # 4 c-tiles
DT = D // 128   # 4 d-tiles
KT = KN // 128  # 16 gemm k-tiles
NT = KN // 512  # 4 gemm n-tiles
GEMM_CHUNK = 4  # concurrent PSUM banks for the big GEMM


def _build(tc, nc, qst_d, ctx_d, simw_d, qacw_d, qacb_d, out_d, scratch):
    from contextlib import ExitStack

    Exp = mybir.ActivationFunctionType.Exp
    AX = mybir.AxisListType.X

    with ExitStack() as ctx:
        consts = ctx.enter_context(tc.tile_pool(name="consts", bufs=1))
        wpool = ctx.enter_context(tc.tile_pool(name="wpool", bufs=1))
        stg = ctx.enter_context(tc.tile_pool(name="stg", bufs=1))
        ab = ctx.enter_context(tc.tile_pool(name="ab", bufs=2))      # A blocks (both batches live)
        b1 = ctx.enter_context(tc.tile_pool(name="b1", bufs=1))      # attention-only tiles
        sm = ctx.enter_context(tc.tile_pool(name="sm", bufs=2))      # small per-batch tiles
        ostage = ctx.enter_context(tc.tile_pool(name="ostage", bufs=1))
        psum_og = ctx.enter_context(tc.tile_pool(name="psum_og", bufs=GEMM_CHUNK, space="PSUM"))
        psum_pw = ctx.enter_context(tc.tile_pool(name="psum_pw", bufs=2, space="PSUM"))
        psum_big = ctx.enter_context(tc.tile_pool(name="psum_big", bufs=1, space="PSUM"))
        psum_at = ctx.enter_context(tc.tile_pool(name="psum_at", bufs=1, space="PSUM"))

        # ---------------- constants ----------------
        ident = consts.tile([128, 128], F32)
        make_identity(nc, ident)
        ident_bf = consts.tile([128, 128], BF16)
        make_identity(nc, ident_bf)
        ones_f32 = consts.tile([1, 128], F32)
        nc.vector.memset(ones_f32, 1.0)
        one_one = consts.tile([1, 1], F32)
        nc.vector.memset(one_one, 1.0)

        simw = simw_d.ap()
        wc_all = consts.tile([128, DT], F32)  # w_c as 4 column tiles
        wq_all = consts.tile([128, DT], F32)
        wp_all = consts.tile([128, DT], F32)
        nc.sync.dma_start(out=wc_all, in_=simw[0, 0:D].rearrange("(k p) -> p k", p=128))
        nc.sync.dma_start(out=wq_all, in_=simw[0, D:2 * D].rearrange("(k p) -> p k", p=128))
        nc.sync.dma_start(out=wp_all, in_=simw[0, 2 * D:3 * D].rearrange("(k p) -> p k", p=128))

        # qac_b broadcast to all 128 partitions via stride-0 DMA
        bias_bc = consts.tile([128, KN], F32)
        qb_ap = qacb_d.ap()
        nc.sync.dma_start(
            out=bias_bc,
            in_=bass.AP(tensor=qb_ap.tensor, offset=qb_ap.offset, ap=[[0, 128]] + qb_ap.ap),
        )

        # ---------------- attention for both batches ----------------
        # (emitted before the weight prologue so its PE work runs while
        #  qac_w streams in from HBM)
        ct = {}        # [b,ci] ctx tiles   [128c, 512d] f32
        ctxT_bf = {}   # [b,di] A block 0   [128d, 512c] bf16
        uT_bf = {}     # [b,di] A block 1
        uc_bf = {}     # [b,di] A block 2
        hc_bf = {}     # [b,di] A block 3

        def emit_attention(b):
            # DMA inputs
            qst = sm.tile([64, D], F32, tag="qst", name=f"qst{b}")
            nc.sync.dma_start(out=qst, in_=qst_d.ap()[b])
            for ci in range(MT):
                t = b1.tile([128, D], F32, tag=f"ct{ci}", name=f"ct{b}_{ci}")
                nc.sync.dma_start(out=t, in_=ctx_d.ap()[b, ci * 128:(ci + 1) * 128, :])
                ct[(b, ci)] = t

            # qstT [D, Q] via PE transpose (4 blocks side by side in one bank)
            p_qt = psum_at.tile([128, DT * 64], F32, tag="at", name=f"pqt{b}")
            for di in range(DT):
                nc.tensor.transpose(
                    p_qt[:, di * 64:(di + 1) * 64],
                    qst[:, di * 128:(di + 1) * 128],
                    ident[0:64, 0:64],
                )
            qstT = sm.tile([128, DT, 64], F32, tag="qstT", name=f"qstT{b}")
            nc.vector.tensor_copy(qstT, p_qt.rearrange("p (d q) -> p d q", d=DT))
            qst_bf = sm.tile([64, D], BF16, tag="qst_bf", name=f"qst_bf{b}")
            nc.scalar.copy(qst_bf, qst)

            # rhs of the sim matmul: [wp*qstT | wc] -> [128, DT, 65]
            wpq = sm.tile([128, DT, 65], F32, tag="wpq", name=f"wpq{b}")
            for di in range(DT):
                nc.vector.tensor_scalar_mul(wpq[:, di, 0:64], qstT[:, di, :], wp_all[:, di:di + 1])
                nc.vector.tensor_copy(wpq[:, di, 64:65], wc_all[:, di:di + 1])

            # ctxT via PE transpose; f32 copy for the sim lhsT, bf16 for A block 0
            ctxT_f32 = {}
            for di in range(DT):
                p_ct = psum_big.tile([128, C], F32, tag="big", name=f"pct{b}_{di}")
                for ci in range(MT):
                    nc.tensor.transpose(
                        p_ct[:, ci * 128:(ci + 1) * 128],
                        ct[(b, ci)][:, di * 128:(di + 1) * 128],
                        ident,
                    )
                tf = b1.tile([128, C], F32, tag=f"ctxT{di}", name=f"ctxT{b}_{di}")
                nc.vector.tensor_copy(tf, p_ct)
                tb = ab.tile([128, C], BF16, tag=f"ctxTb{di}", name=f"ctxTb{b}_{di}")
                nc.vector.tensor_copy(tb, tf)
                ctxT_f32[di] = tf
                ctxT_bf[(b, di)] = tb

            # qw row: qw[q] = qst[q,:] @ w_q  (65th col stays 0 so cw is untouched)
            p_qw = psum_at.tile([1, 64], F32, tag="at", name=f"pqw{b}")
            for di in range(DT):
                nc.tensor.matmul(
                    p_qw, wq_all[:, di:di + 1], qstT[:, di, :],
                    start=(di == 0), stop=(di == DT - 1),
                )
            qw_row = sm.tile([1, 65], F32, tag="qw_row", name=f"qw_row{b}")
            nc.vector.memset(qw_row[0:1, 64:65], 0.0)
            nc.vector.tensor_copy(qw_row[0:1, 0:64], p_qw)

            # sim' cols 0:64 (+ cw in col 64) = ctxT.T @ [wp*qstT | wc] + [qw | 0]
            p_sim = psum_at.tile([128, MT, 65], F32, tag="at", name=f"psim{b}")
            for mi in range(MT):
                for di in range(DT):
                    nc.tensor.matmul(
                        p_sim[:, mi, :],
                        ctxT_f32[di][:, mi * 128:(mi + 1) * 128],
                        wpq[:, di, :],
                        start=(di == 0), stop=False,
                    )
                nc.tensor.matmul(
                    p_sim[:, mi, :], ones_f32, qw_row,
                    start=False, stop=True,
                )

            # row softmax over q (free axis); keep neg-rowmax for the attn path
            negm = sm.tile([128, MT], F32, tag="negm", name=f"negm{b}")
            ssum = sm.tile([128, MT], F32, tag="ssum", name=f"ssum{b}")
            rsum = sm.tile([128, MT], F32, tag="rsum", name=f"rsum{b}")
            S_n = sm.tile([128, MT, 64], F32, tag="S_n", name=f"S_n{b}")
            g_col = sm.tile([128, MT], F32, tag="g_col", name=f"g_col{b}")
            for mi in range(MT):
                nc.vector.reduce_max(negm[:, mi:mi + 1], p_sim[:, mi, 0:64], axis=AX, negate=True)
                nc.scalar.activation(
                    S_n[:, mi, :], p_sim[:, mi, 0:64], Exp,
                    bias=negm[:, mi:mi + 1], accum_out=ssum[:, mi:mi + 1],
                )
                # g[c] = max_q sim' + cw = cw - negm   (cw lives in sim col 64)
                nc.vector.tensor_sub(g_col[:, mi:mi + 1], p_sim[:, mi, 64:65], negm[:, mi:mi + 1])
            nc.vector.reciprocal(rsum, ssum)
            for mi in range(MT):
                nc.vector.tensor_scalar_mul(S_n[:, mi, :], S_n[:, mi, :], rsum[:, mi:mi + 1])

            # S_T [Q, C] (bf16) then u_T = qst.T @ S_T  -> [D, C]
            p_st = psum_at.tile([64, C], F32, tag="at", name=f"pst{b}")
            for mi in range(MT):
                nc.tensor.transpose(
                    p_st[:, mi * 128:(mi + 1) * 128], S_n[:, mi, :], ident,
                )
            S_T = sm.tile([64, C], BF16, tag="S_T", name=f"S_T{b}")
            nc.vector.tensor_copy(S_T, p_st)

            for di in range(DT):
                p_u = psum_big.tile([128, C], F32, tag="big", name=f"pu{b}_{di}")
                nc.tensor.matmul(
                    p_u, qst_bf[:, di * 128:(di + 1) * 128], S_T,
                    start=True, stop=True,
                )
                tu = ab.tile([128, C], BF16, tag=f"uT{di}", name=f"uT{b}_{di}")
                nc.vector.tensor_copy(tu, p_u)
                uT_bf[(b, di)] = tu
                tuc = ab.tile([128, C], BF16, tag=f"uc{di}", name=f"uc{b}_{di}")
                nc.vector.tensor_mul(tuc, tu, ctxT_bf[(b, di)])
                uc_bf[(b, di)] = tuc

            # attn = softmax_c(g) on one partition
            p_g = psum_at.tile([1, C], F32, tag="at", name=f"pg{b}")
            for mi in range(MT):
                nc.tensor.transpose(
                    p_g[0:1, mi * 128:(mi + 1) * 128], g_col[:, mi:mi + 1], ident,
                )
            g_row = sm.tile([1, C], F32, tag="g_row", name=f"g_row{b}", bufs=1)
            nc.vector.tensor_copy(g_row, p_g)
            negm1 = sm.tile([1, 1], F32, tag="negm1", name=f"negm1{b}")
            ssum1 = sm.tile([1, 1], F32, tag="ssum1", name=f"ssum1{b}")
            nc.vector.reduce_max(negm1, g_row, axis=AX, negate=True)
            attn_row = sm.tile([1, C], F32, tag="attn_row", name=f"attn_row{b}", bufs=1)
            nc.scalar.activation(attn_row, g_row, Exp, bias=negm1, accum_out=ssum1)
            rsum1 = sm.tile([1, 1], F32, tag="rsum1", name=f"rsum1{b}")
            nc.vector.reciprocal(rsum1, ssum1)
            nc.vector.tensor_scalar_mul(attn_row, attn_row, rsum1)

            # attn column [C,1]; h_row = attn @ ctx; h column [D,1]
            p_ac = psum_at.tile([128, MT], F32, tag="at", name=f"pac{b}")
            for mi in range(MT):
                nc.tensor.matmul(
                    p_ac[:, mi:mi + 1],
                    attn_row[0:1, mi * 128:(mi + 1) * 128], one_one,
                    start=True, stop=True,
                )
            ac = sm.tile([128, MT], F32, tag="ac", name=f"ac{b}")
            nc.vector.tensor_copy(ac, p_ac)

            p_hr = psum_at.tile([1, D], F32, tag="at", name=f"phr{b}")
            for ci in range(MT):
                nc.tensor.matmul(
                    p_hr, ac[:, ci:ci + 1], ct[(b, ci)],
                    start=(ci == 0), stop=(ci == MT - 1),
                )
            h_row = sm.tile([1, D], F32, tag="h_row", name=f"h_row{b}", bufs=1)
            nc.vector.tensor_copy(h_row, p_hr)

            p_hc = psum_at.tile([128, DT], F32, tag="at", name=f"phc{b}")
            for di in range(DT):
                nc.tensor.matmul(
                    p_hc[:, di:di + 1],
                    h_row[0:1, di * 128:(di + 1) * 128], one_one,
                    start=True, stop=True,
                )
            h_col = sm.tile([128, DT], F32, tag="h_col", name=f"h_col{b}")
            nc.vector.tensor_copy(h_col, p_hc)

            for di in range(DT):
                th = ab.tile([128, C], BF16, tag=f"hc{di}", name=f"hc{b}_{di}")
                nc.vector.tensor_scalar_mul(th, ctxT_bf[(b, di)], h_col[:, di:di + 1])
                hc_bf[(b, di)] = th

        # -------- qac_w.T prologue (column strips, PE transposes in bf16) --------
        # interleaved with GEMM chunk 0 so the PE warms up and the GEMM starts as
        # soon as the first k-tiles of W^T are ready.
        WT = [wpool.tile([128, KN], BF16, tag=f"wt{k}", name=f"wt{k}") for k in range(KT)]

        # one chunk per (batch, c-tile): the 4 matmuls of each k-layer share the
        # same stationary operand (one LDWEIGHTS per layer), and the drain is a
        # single contiguous [128, 2048] store.
        groups = [(b, mi) for b in range(BPC) for mi in range(MT)]

        def ablk(b, kt):
            if kt < 4:
                return ctxT_bf[(b, kt)]
            if kt < 8:
                return uT_bf[(b, kt - 4)]
            if kt < 12:
                return uc_bf[(b, kt - 8)]
            return hc_bf[(b, kt - 12)]

        def gemm_layer(bmi, psums, kt):
            b, mi = bmi
            for ni in range(NT):
                nc.tensor.matmul(
                    psums[ni],
                    ablk(b, kt)[:, mi * 128:(mi + 1) * 128],
                    WT[kt][:, ni * 512:(ni + 1) * 512],
                    start=(kt == 0), stop=(kt == KT - 1),
                )

        def gemm_drain(bmi, psums, c0):
            b, mi = bmi
            st = ostage.tile([128, KN], F32, tag="ost", name=f"ost{c0}")
            for ni in range(NT):
                nc.vector.tensor_add(
                    st[:, ni * 512:(ni + 1) * 512], psums[ni],
                    bias_bc[:, ni * 512:(ni + 1) * 512],
                )
            nc.sync.dma_start(out=out_d.ap()[b, mi * 128:(mi + 1) * 128, :], in_=st)

        chunk0 = groups[0]
        psums0 = [
            psum_og.tile([128, 512], F32, tag="og", name=f"po0_{gi}")
            for gi in range(NT)
        ]

        # Column halves: load [128,1024] f32 strips of qac_w, cast to a bf16
        # row cache, transpose in a dense burst. Phase order puts batch-0
        # attention before half 0 and batch-1 attention between the halves so
        # every engine has fill-in work while qac_w streams from HBM.
        HW = KN // 2

        def qac_load_half(half):
            rbf = []
            for ri in range(KT):
                t = stg.tile([128, HW], F32, tag=f"qs{ri % 3}", name=f"qs{half}_{ri}")
                nc.sync.dma_start(
                    out=t,
                    in_=qacw_d.ap()[ri * 128:(ri + 1) * 128, half * HW:(half + 1) * HW],
                )
                tb = stg.tile([128, HW], BF16, tag=f"qsb{ri}", name=f"qsb{half}_{ri}")
                nc.vector.tensor_copy(tb, t)
                rbf.append(tb)
            return rbf

        def qac_transpose_half(half, rbf):
            for ktl in range(8):
                kt = half * 8 + ktl
                for rg in range(4):
                    p_w = psum_pw.tile([128, 512], BF16, tag="pw", name=f"pw{kt}_{rg}")
                    for rr in range(4):
                        nc.tensor.transpose(
                            p_w[:, rr * 128:(rr + 1) * 128],
                            rbf[rg * 4 + rr][:, ktl * 128:(ktl + 1) * 128],
                            ident_bf,
                        )
                    if rg % 2 == 0:
                        nc.vector.tensor_copy(WT[kt][:, rg * 512:(rg + 1) * 512], p_w)
                    else:
                        nc.scalar.copy(WT[kt][:, rg * 512:(rg + 1) * 512], p_w)

        emit_attention(0)
        rbf0 = qac_load_half(0)
        qac_transpose_half(0, rbf0)
        for ktl in range(8):
            gemm_layer(chunk0, psums0, ktl)
        emit_attention(1)
        rbf1 = qac_load_half(1)
        qac_transpose_half(1, rbf1)
        for ktl in range(8):
            gemm_layer(chunk0, psums0, 8 + ktl)
        gemm_drain(chunk0, psums0, 0)

        # ---------------- remaining GEMM chunks ----------------
        for c0 in range(1, len(groups)):
            chunk = groups[c0]
            psums = [
                psum_og.tile([128, 512], F32, tag="og", name=f"po{c0}_{gi}")
                for gi in range(NT)
            ]
            for kt in range(KT):
                gemm_layer(chunk, psums, kt)
            gemm_drain(chunk, psums, c0)


def build_nc():
    nc = bacc.Bacc("TRN2", target_bir_lowering=False, debug=False, num_devices=NCORES)
    qst_d = nc.declare_dram_parameter("questions", [BPC, Q, D], F32, isOutput=False)
    ctx_d = nc.declare_dram_parameter("contexts", [BPC, C, D], F32, isOutput=False)
    simw_d = nc.declare_dram_parameter("sim_w", [1, 3 * D], F32, isOutput=False)
    qacw_d = nc.declare_dram_parameter("qac_w", [KN, KN], F32, isOutput=False)
    qacb_d = nc.declare_dram_parameter("qac_b", [KN], F32, isOutput=False)
    out_d = nc.declare_dram_parameter("out", [BPC, C, KN], F32, isOutput=True)
    scratch = nc.dram_tensor("qac_bf16_h1", [KN, KN // 2], mybir.dt.bfloat16)
    with tile.TileContext(nc) as tc:
        _build(tc, nc, qst_d, ctx_d, simw_d, qacw_d, qacb_d, out_d, scratch)
    nc.compile()
    return nc


def shard_inputs(questions, contexts, sim_w, qac_w, qac_b):
    questions = np.ascontiguousarray(np.asarray(questions, dtype=np.float32))
    contexts = np.ascontiguousarray(np.asarray(contexts, dtype=np.float32))
    sim_w = np.ascontiguousarray(np.asarray(sim_w, dtype=np.float32))
    qac_w = np.ascontiguousarray(np.asarray(qac_w, dtype=np.float32))
    qac_b = np.ascontiguousarray(np.asarray(qac_b, dtype=np.float32))
    return [
        {
            "questions": questions[i * BPC:(i + 1) * BPC],
            "contexts": contexts[i * BPC:(i + 1) * BPC],
            "sim_w": sim_w,
            "qac_w": qac_w,
            "qac_b": qac_b,
        }
        for i in range(NCORES)
    ]


_NC = None


def get_nc():
    global _NC
    if _NC is None:
        _NC = build_nc()
    return _NC


def kernel(questions, contexts, sim_w, qac_w, qac_b):
    from concourse.bass_utils import run_bass_kernel_spmd

    nc = get_nc()
    in_maps = shard_inputs(questions, contexts, sim_w, qac_w, qac_b)
    res = run_bass_kernel_spmd(nc, in_maps, core_ids=list(range(NCORES)))
    return np.concatenate([res.results[i]["out"] for i in range(NCORES)], axis=0)


# revision 16
# speedup vs baseline: 1.1012x; 1.0071x over previous
"""AttentionFlow (BiDAF-style) Trainium2 kernel, data-parallel over batch on 8 cores.

Shapes (hardcoded): B=16, Q=64, C=512, D=512; final GEMM [C,4D]@[4D,4D].
Each core handles 2 batches independently; no collectives.

Per-batch math (matching the reference):
  sim'[c,q] = (ctx*w_p)[c,:]@qst[q,:] + qw[q]        (cw[c] cancels in softmax over q)
  S = softmax_q(sim'); u = S @ qst
  attn = softmax_c(max_q sim' + cw);  h = attn @ ctx
  A = [ctx, u, u*ctx, h*ctx]  (built transposed, [4D, C])
  out = A @ qac_w.T + qac_b

qac_w.T is built with TensorE transposes in bf16, column-strip order, with the
first GEMM chunk's matmuls interleaved so the PE stays warm and the GEMM starts
as soon as the first k-tiles of W^T land. cw rides as a 65th rhs column of the
sim matmul; qac_b is broadcast by a stride-0 DMA and added during the PSUM
drain on the vector engine.
"""

import numpy as np

import concourse.bass as bass
import concourse.mybir as mybir
import concourse.tile as tile
from concourse import bacc
from concourse.masks import make_identity

B, Q, C, D = 16, 64, 512, 512
KN = 4 * D  # 2048
NCORES = 8
BPC = B // NCORES  # 2 batches per core

F32 = mybir.dt.float32
BF16 = mybir.dt.bfloat16

MT = C // 128   # 4 c-tiles
DT = D // 128   # 4 d-tiles
KT = KN // 128  # 16 gemm k-tiles
NT = KN // 512  # 4 gemm n-tiles
GEMM_CHUNK = 4  # concurrent PSUM banks for the big GEMM


def _build(tc, nc, qst_d, ctx_d, simw_d, qacw_d, qacb_d, out_d, scratch):
    from contextlib import ExitStack

    Exp = mybir.ActivationFunctionType.Exp
    AX = mybir.AxisListType.X

    with ExitStack() as ctx:
        consts = ctx.enter_context(tc.tile_pool(name="consts", bufs=1))
        wpool = ctx.enter_context(tc.tile_pool(name="wpool", bufs=1))
        stg = ctx.enter_context(tc.tile_pool(name="stg", bufs=1))
        ab = ctx.enter_context(tc.tile_pool(name="ab", bufs=2))      # A blocks (both batches live)
        b1 = ctx.enter_context(tc.tile_pool(name="b1", bufs=1))      # attention-only tiles
        sm = ctx.enter_context(tc.tile_pool(name="sm", bufs=2))      # small per-batch tiles
        ostage = ctx.enter_context(tc.tile_pool(name="ostage", bufs=1))
        psum_og = ctx.enter_context(tc.tile_pool(name="psum_og", bufs=6, space="PSUM"))
        psum_big = ctx.enter_context(tc.tile_pool(name="psum_big", bufs=1, space="PSUM"))
        psum_at = ctx.enter_context(tc.tile_pool(name="psum_at", bufs=1, space="PSUM"))

        # ---------------- constants ----------------
        ident = consts.tile([128, 128], F32)
        make_identity(nc, ident)
        ident_bf = consts.tile([128, 128], BF16)
        make_identity(nc, ident_bf)
        ones_f32 = consts.tile([1, 128], F32)
        nc.vector.memset(ones_f32, 1.0)
        one_one = consts.tile([1, 1], F32)
        nc.vector.memset(one_one, 1.0)

        simw = simw_d.ap()
        wc_all = consts.tile([128, DT], F32)  # w_c as 4 column tiles
        wq_all = consts.tile([128, DT], F32)
        wp_all = consts.tile([128, DT], F32)
        nc.sync.dma_start(out=wc_all, in_=simw[0, 0:D].rearrange("(k p) -> p k", p=128))
        nc.sync.dma_start(out=wq_all, in_=simw[0, D:2 * D].rearrange("(k p) -> p k", p=128))
        nc.sync.dma_start(out=wp_all, in_=simw[0, 2 * D:3 * D].rearrange("(k p) -> p k", p=128))

        # qac_b broadcast to all 128 partitions via stride-0 DMA
        bias_bc = consts.tile([128, KN], F32)
        qb_ap = qacb_d.ap()
        nc.sync.dma_start(
            out=bias_bc,
            in_=bass.AP(tensor=qb_ap.tensor, offset=qb_ap.offset, ap=[[0, 128]] + qb_ap.ap),
        )

        # ---------------- attention for both batches ----------------
        # (emitted before the weight prologue so its PE work runs while
        #  qac_w streams in from HBM)
        ct = {}        # [b,ci] ctx tiles   [128c, 512d] f32
        ctxT_bf = {}   # [b,di] A block 0   [128d, 512c] bf16
        uT_bf = {}     # [b,di] A block 1
        uc_bf = {}     # [b,di] A block 2
        hc_bf = {}     # [b,di] A block 3

        def emit_attention(b):
            # DMA inputs
            qst = sm.tile([64, D], F32, tag="qst", name=f"qst{b}")
            nc.sync.dma_start(out=qst, in_=qst_d.ap()[b])
            for ci in range(MT):
                t = b1.tile([128, D], F32, tag=f"ct{ci}", name=f"ct{b}_{ci}")
                nc.sync.dma_start(out=t, in_=ctx_d.ap()[b, ci * 128:(ci + 1) * 128, :])
                ct[(b, ci)] = t

            # qstT [D, Q] via PE transpose (4 blocks side by side in one bank)
            p_qt = psum_at.tile([128, DT * 64], F32, tag="at", name=f"pqt{b}")
            for di in range(DT):
                nc.tensor.transpose(
                    p_qt[:, di * 64:(di + 1) * 64],
                    qst[:, di * 128:(di + 1) * 128],
                    ident[0:64, 0:64],
                )
            qstT = sm.tile([128, DT, 64], F32, tag="qstT", name=f"qstT{b}")
            nc.vector.tensor_copy(qstT, p_qt.rearrange("p (d q) -> p d q", d=DT))
            qst_bf = sm.tile([64, D], BF16, tag="qst_bf", name=f"qst_bf{b}")
            nc.scalar.copy(qst_bf, qst)

            # rhs of the sim matmul: [wp*qstT | wc] -> [128, DT, 65]
            wpq = sm.tile([128, DT, 65], F32, tag="wpq", name=f"wpq{b}")
            for di in range(DT):
                nc.vector.tensor_scalar_mul(wpq[:, di, 0:64], qstT[:, di, :], wp_all[:, di:di + 1])
                nc.vector.tensor_copy(wpq[:, di, 64:65], wc_all[:, di:di + 1])

            # ctxT via PE transpose; f32 copy for the sim lhsT, bf16 for A block 0
            ctxT_f32 = {}
            for di in range(DT):
                p_ct = psum_big.tile([128, C], F32, tag="big", name=f"pct{b}_{di}")
                for ci in range(MT):
                    nc.tensor.transpose(
                        p_ct[:, ci * 128:(ci + 1) * 128],
                        ct[(b, ci)][:, di * 128:(di + 1) * 128],
                        ident,
                    )
                tf = b1.tile([128, C], F32, tag=f"ctxT{di}", name=f"ctxT{b}_{di}")
                nc.vector.tensor_copy(tf, p_ct)
                tb = ab.tile([128, C], BF16, tag=f"ctxTb{di}", name=f"ctxTb{b}_{di}")
                nc.vector.tensor_copy(tb, tf)
                ctxT_f32[di] = tf
                ctxT_bf[(b, di)] = tb

            # qw row: qw[q] = qst[q,:] @ w_q  (65th col stays 0 so cw is untouched)
            p_qw = psum_at.tile([1, 64], F32, tag="at", name=f"pqw{b}")
            for di in range(DT):
                nc.tensor.matmul(
                    p_qw, wq_all[:, di:di + 1], qstT[:, di, :],
                    start=(di == 0), stop=(di == DT - 1),
                )
            qw_row = sm.tile([1, 65], F32, tag="qw_row", name=f"qw_row{b}")
            nc.vector.memset(qw_row[0:1, 64:65], 0.0)
            nc.vector.tensor_copy(qw_row[0:1, 0:64], p_qw)

            # sim' cols 0:64 (+ cw in col 64) = ctxT.T @ [wp*qstT | wc] + [qw | 0]
            p_sim = psum_at.tile([128, MT, 65], F32, tag="at", name=f"psim{b}")
            for mi in range(MT):
                for di in range(DT):
                    nc.tensor.matmul(
                        p_sim[:, mi, :],
                        ctxT_f32[di][:, mi * 128:(mi + 1) * 128],
                        wpq[:, di, :],
                        start=(di == 0), stop=False,
                    )
                nc.tensor.matmul(
                    p_sim[:, mi, :], ones_f32, qw_row,
                    start=False, stop=True,
                )

            # row softmax over q (free axis); keep neg-rowmax for the attn path
            negm = sm.tile([128, MT], F32, tag="negm", name=f"negm{b}")
            ssum = sm.tile([128, MT], F32, tag="ssum", name=f"ssum{b}")
            rsum = sm.tile([128, MT], F32, tag="rsum", name=f"rsum{b}")
            S_n = sm.tile([128, MT, 64], F32, tag="S_n", name=f"S_n{b}")
            g_col = sm.tile([128, MT], F32, tag="g_col", name=f"g_col{b}")
            for mi in range(MT):
                nc.vector.reduce_max(negm[:, mi:mi + 1], p_sim[:, mi, 0:64], axis=AX, negate=True)
                nc.scalar.activation(
                    S_n[:, mi, :], p_sim[:, mi, 0:64], Exp,
                    bias=negm[:, mi:mi + 1], accum_out=ssum[:, mi:mi + 1],
                )
                # g[c] = max_q sim' + cw = cw - negm   (cw lives in sim col 64)
                nc.vector.tensor_sub(g_col[:, mi:mi + 1], p_sim[:, mi, 64:65], negm[:, mi:mi + 1])
            nc.vector.reciprocal(rsum, ssum)
            for mi in range(MT):
                nc.vector.tensor_scalar_mul(S_n[:, mi, :], S_n[:, mi, :], rsum[:, mi:mi + 1])

            # S_T [Q, C] (bf16) then u_T = qst.T @ S_T  -> [D, C]
            p_st = psum_at.tile([64, C], F32, tag="at", name=f"pst{b}")
            for mi in range(MT):
                nc.tensor.transpose(
                    p_st[:, mi * 128:(mi + 1) * 128], S_n[:, mi, :], ident,
                )
            S_T = sm.tile([64, C], BF16, tag="S_T", name=f"S_T{b}")
            nc.vector.tensor_copy(S_T, p_st)

            for di in range(DT):
                p_u = psum_big.tile([128, C], F32, tag="big", name=f"pu{b}_{di}")
                nc.tensor.matmul(
                    p_u, qst_bf[:, di * 128:(di + 1) * 128], S_T,
                    start=True, stop=True,
                )
                tu = ab.tile([128, C], BF16, tag=f"uT{di}", name=f"uT{b}_{di}")
                nc.vector.tensor_copy(tu, p_u)
                uT_bf[(b, di)] = tu
                tuc = ab.tile([128, C], BF16, tag=f"uc{di}", name=f"uc{b}_{di}")
                nc.vector.tensor_mul(tuc, tu, ctxT_bf[(b, di)])
                uc_bf[(b, di)] = tuc

            # attn = softmax_c(g) on one partition
            p_g = psum_at.tile([1, C], F32, tag="at", name=f"pg{b}")
            for mi in range(MT):
                nc.tensor.transpose(
                    p_g[0:1, mi * 128:(mi + 1) * 128], g_col[:, mi:mi + 1], ident,
                )
            g_row = sm.tile([1, C], F32, tag="g_row", name=f"g_row{b}", bufs=1)
            nc.vector.tensor_copy(g_row, p_g)
            negm1 = sm.tile([1, 1], F32, tag="negm1", name=f"negm1{b}")
            ssum1 = sm.tile([1, 1], F32, tag="ssum1", name=f"ssum1{b}")
            nc.vector.reduce_max(negm1, g_row, axis=AX, negate=True)
            attn_row = sm.tile([1, C], F32, tag="attn_row", name=f"attn_row{b}", bufs=1)
            nc.scalar.activation(attn_row, g_row, Exp, bias=negm1, accum_out=ssum1)
            rsum1 = sm.tile([1, 1], F32, tag="rsum1", name=f"rsum1{b}")
            nc.vector.reciprocal(rsum1, ssum1)
            nc.vector.tensor_scalar_mul(attn_row, attn_row, rsum1)

            # attn column [C,1]; h_row = attn @ ctx; h column [D,1]
            p_ac = psum_at.tile([128, MT], F32, tag="at", name=f"pac{b}")
            for mi in range(MT):
                nc.tensor.matmul(
                    p_ac[:, mi:mi + 1],
                    attn_row[0:1, mi * 128:(mi + 1) * 128], one_one,
                    start=True, stop=True,
                )
            ac = sm.tile([128, MT], F32, tag="ac", name=f"ac{b}")
            nc.vector.tensor_copy(ac, p_ac)

            p_hr = psum_at.tile([1, D], F32, tag="at", name=f"phr{b}")
            for ci in range(MT):
                nc.tensor.matmul(
                    p_hr, ac[:, ci:ci + 1], ct[(b, ci)],
                    start=(ci == 0), stop=(ci == MT - 1),
                )
            h_row = sm.tile([1, D], F32, tag="h_row", name=f"h_row{b}", bufs=1)
            nc.vector.tensor_copy(h_row, p_hr)

            p_hc = psum_at.tile([128, DT], F32, tag="at", name=f"phc{b}")
            for di in range(DT):
                nc.tensor.matmul(
                    p_hc[:, di:di + 1],
                    h_row[0:1, di * 128:(di + 1) * 128], one_one,
                    start=True, stop=True,
                )
            h_col = sm.tile([128, DT], F32, tag="h_col", name=f"h_col{b}")
            nc.vector.tensor_copy(h_col, p_hc)

            for di in range(DT):
                th = ab.tile([128, C], BF16, tag=f"hc{di}", name=f"hc{b}_{di}")
                nc.vector.tensor_scalar_mul(th, ctxT_bf[(b, di)], h_col[:, di:di + 1])
                hc_bf[(b, di)] = th

        # -------- qac_w.T prologue (column strips, PE transposes in bf16) --------
        # interleaved with GEMM chunk 0 so the PE warms up and the GEMM starts as
        # soon as the first k-tiles of W^T are ready.
        WT = [wpool.tile([128, KN], BF16, tag=f"wt{k}", name=f"wt{k}") for k in range(KT)]

        # one chunk per (batch, c-tile): the 4 matmuls of each k-layer share the
        # same stationary operand (one LDWEIGHTS per layer), and the drain is a
        # single contiguous [128, 2048] store.
        groups = [(b, mi) for b in range(BPC) for mi in range(MT)]

        def ablk(b, kt):
            if kt < 4:
                return ctxT_bf[(b, kt)]
            if kt < 8:
                return uT_bf[(b, kt - 4)]
            if kt < 12:
                return uc_bf[(b, kt - 8)]
            return hc_bf[(b, kt - 12)]

        def gemm_layer(bmi, psums, kt):
            b, mi = bmi
            for ni in range(NT):
                nc.tensor.matmul(
                    psums[ni],
                    ablk(b, kt)[:, mi * 128:(mi + 1) * 128],
                    WT[kt][:, ni * 512:(ni + 1) * 512],
                    start=(kt == 0), stop=(kt == KT - 1),
                )

        def gemm_drain(bmi, psums, c0):
            b, mi = bmi
            st = ostage.tile([128, KN], F32, tag="ost", name=f"ost{c0}")
            for ni in range(NT):
                nc.vector.tensor_add(
                    st[:, ni * 512:(ni + 1) * 512], psums[ni],
                    bias_bc[:, ni * 512:(ni + 1) * 512],
                )
            nc.sync.dma_start(out=out_d.ap()[b, mi * 128:(mi + 1) * 128, :], in_=st)

        chunk0 = groups[0]
        psums0 = [
            psum_og.tile([128, 512], F32, tag="og", name=f"po0_{gi}")
            for gi in range(NT)
        ]

        # Column halves: load [128,1024] f32 strips of qac_w, cast to a bf16
        # row cache, transpose in a dense burst. Phase order puts batch-0
        # attention before half 0 and batch-1 attention between the halves so
        # every engine has fill-in work while qac_w streams from HBM.
        HW = KN // 2

        def qac_load_half(half):
            rbf = []
            for ri in range(KT):
                t = stg.tile([128, HW], F32, tag=f"qs{ri % 3}", name=f"qs{half}_{ri}")
                nc.sync.dma_start(
                    out=t,
                    in_=qacw_d.ap()[ri * 128:(ri + 1) * 128, half * HW:(half + 1) * HW],
                )
                tb = stg.tile([128, HW], BF16, tag=f"qsb{ri}", name=f"qsb{half}_{ri}")
                nc.vector.tensor_copy(tb, t)
                rbf.append(tb)
            return rbf

        def qac_transpose_half(half, rbf):
            for ktl in range(8):
                kt = half * 8 + ktl
                for rg in range(4):
                    p_w = psum_og.tile([128, 512], BF16, tag="og", name=f"pw{kt}_{rg}")
                    for rr in range(4):
                        nc.tensor.transpose(
                            p_w[:, rr * 128:(rr + 1) * 128],
                            rbf[rg * 4 + rr][:, ktl * 128:(ktl + 1) * 128],
                            ident_bf,
                        )
                    if rg % 2 == 0:
                        nc.vector.tensor_copy(WT[kt][:, rg * 512:(rg + 1) * 512], p_w)
                    else:
                        nc.scalar.copy(WT[kt][:, rg * 512:(rg + 1) * 512], p_w)

        emit_attention(0)
        rbf0 = qac_load_half(0)
        qac_transpose_half(0, rbf0)
        for ktl in range(8):
            gemm_layer(chunk0, psums0, ktl)
        emit_attention(1)
        rbf1 = qac_load_half(1)
        qac_transpose_half(1, rbf1)
        for ktl in range(8):
            gemm_layer(chunk0, psums0, 8 + ktl)
        gemm_drain(chunk0, psums0, 0)

        # ---------------- remaining GEMM chunks ----------------
        for c0 in range(1, len(groups)):
            chunk = groups[c0]
            psums = [
                psum_og.tile([128, 512], F32, tag="og", name=f"po{c0}_{gi}")
                for gi in range(NT)
            ]
            for kt in range(KT):
                gemm_layer(chunk, psums, kt)
            gemm_drain(chunk, psums, c0)


def build_nc():
    nc = bacc.Bacc("TRN2", target_bir_lowering=False, debug=False, num_devices=NCORES)
    qst_d = nc.declare_dram_parameter("questions", [BPC, Q, D], F32, isOutput=False)
    ctx_d = nc.declare_dram_parameter("contexts", [BPC, C, D], F32, isOutput=False)
    simw_d = nc.declare_dram_parameter("sim_w", [1, 3 * D], F32, isOutput=False)
    qacw_d = nc.declare_dram_parameter("qac_w", [KN, KN], F32, isOutput=False)
    qacb_d = nc.declare_dram_parameter("qac_b", [KN], F32, isOutput=False)
    out_d = nc.declare_dram_parameter("out", [BPC, C, KN], F32, isOutput=True)
    scratch = nc.dram_tensor("qac_bf16_h1", [KN, KN // 2], mybir.dt.bfloat16)
    with tile.TileContext(nc) as tc:
        _build(tc, nc, qst_d, ctx_d, simw_d, qacw_d, qacb_d, out_d, scratch)
    nc.compile()
    return nc


def shard_inputs(questions, contexts, sim_w, qac_w, qac_b):
    questions = np.ascontiguousarray(np.asarray(questions, dtype=np.float32))
    contexts = np.ascontiguousarray(np.asarray(contexts, dtype=np.float32))
    sim_w = np.ascontiguousarray(np.asarray(sim_w, dtype=np.float32))
    qac_w = np.ascontiguousarray(np.asarray(qac_w, dtype=np.float32))
    qac_b = np.ascontiguousarray(np.asarray(qac_b, dtype=np.float32))
    return [
        {
            "questions": questions[i * BPC:(i + 1) * BPC],
            "contexts": contexts[i * BPC:(i + 1) * BPC],
            "sim_w": sim_w,
            "qac_w": qac_w,
            "qac_b": qac_b,
        }
        for i in range(NCORES)
    ]


_NC = None


def get_nc():
    global _NC
    if _NC is None:
        _NC = build_nc()
    return _NC


def kernel(questions, contexts, sim_w, qac_w, qac_b):
    from concourse.bass_utils import run_bass_kernel_spmd

    nc = get_nc()
    in_maps = shard_inputs(questions, contexts, sim_w, qac_w, qac_b)
    res = run_bass_kernel_spmd(nc, in_maps, core_ids=list(range(NCORES)))
    return np.concatenate([res.results[i]["out"] for i in range(NCORES)], axis=0)
